# revision 1
# baseline (speedup 1.0000x reference)
"""TRN2 Bass kernel for sliding-window causal GQA attention block.

Reference computation (B=2, T=2048, C=2048, NH=16, NKV=4, HD=128, WIN=512):
  qkv = x @ w_qkv.T ; RoPE(q, k) ; GQA repeat ; banded causal attention
  (keys j in [i-511, i]) ; y @ w_proj.T

Sharding: 8 cores = batch (2) x kv-head-group (4) tensor parallel.
Core c = b*4+g owns batch b, q-heads [4g..4g+4), kv head g. Each core
computes a partial output (contribution of its 512 y-dims to all 2048
out dims); host sums the 4 partials per batch.

v2 layout notes:
 - everything on-chip is bf16 except PSUM accumulation (fp32).
 - scores are computed transposed (keys on partitions, queries on the
   free axis) so the exp'd probs tile can feed the PV matmul directly
   as the stationary operand: y[q,d] = sum_k probsT[k,q] * v[k,d].
   This removes all probs transposes and PSUM->SBUF copies.
 - v carries an appended ones-column, so the PV accumulation yields the
   softmax denominator for free in column HD.
 - causal/window masks are folded into the score accumulation group on
   the tensor engine via identity-matmul (PSUM += I.T @ M).
 - DMAs are coalesced (one per 512-token chunk for x and out).
"""
import sys
sys.path.insert(0, '/opt/trn_rl_repo')
import numpy as np
import ml_dtypes

import concourse.bass as bass
from concourse import bacc
import concourse.tile as tile
from concourse import mybir
from concourse.bass_utils import run_bass_kernel_spmd
from concourse.masks import make_identity

T = 2048
C = 2048
HD = 128
NH = 16
NKV = 4
NQL = 4           # q heads per core
WIN = 512
QKVF = NQL * HD + 2 * HD   # 768 local qkv features
SCALE = float(1.0 / np.sqrt(HD))
QB = T // 128     # 16 q/k blocks
KC = C // 128     # 16 contraction tiles
NCH = T // 512    # 4 token chunks
VW = 130          # v tile width: HD cols of v + ones col at HD (+pad)
NEG = -1e9

f32 = mybir.dt.float32
bf16 = mybir.dt.bfloat16

_CACHE = {}


def _build_program():
    nc = bacc.Bacc()
    xT = nc.declare_dram_parameter("xT", [C, T], bf16, isOutput=False)
    wqkvT = nc.declare_dram_parameter("wqkvT", [C, QKVF], bf16, isOutput=False)
    wpT = nc.declare_dram_parameter("wpT", [NQL * HD, C], bf16, isOutput=False)
    cosE = nc.declare_dram_parameter("cosE", [HD, T], bf16, isOutput=False)
    sinE = nc.declare_dram_parameter("sinE", [HD, T], bf16, isOutput=False)
    mdT = nc.declare_dram_parameter("mdT", [128, 128], bf16, isOutput=False)
    mwT = nc.declare_dram_parameter("mwT", [128, 128], bf16, isOutput=False)
    rotP = nc.declare_dram_parameter("rotP", [128, 128], bf16, isOutput=False)
    outT = nc.declare_dram_parameter("outT", [C, T], bf16, isOutput=True)

    import os as _os
    _dbg = _os.environ.get("KDBG", "0") == "1"
    if _dbg:
        qkd = nc.declare_dram_parameter("qkd", [5 * HD, T], bf16, isOutput=True)
        yTd = nc.declare_dram_parameter("yTd", [NQL * HD, T], bf16, isOutput=True)
        prd = nc.declare_dram_parameter("prd", [128, 640], bf16, isOutput=True)
        vd = nc.declare_dram_parameter("vd", [128, VW], bf16, isOutput=True)
    _tsim = _os.environ.get("KERNEL_TRACE_SIM", "0") == "1"
    with tile.TileContext(nc, trace_sim=_tsim) as tc:
        with tc.tile_pool(name="persist", bufs=1) as persist, \
             tc.tile_pool(name="qkv_out", bufs=1) as qkv_out, \
             tc.tile_pool(name="ytile", bufs=1) as ytile, \
             tc.tile_pool(name="outst", bufs=3) as outst, \
             tc.tile_pool(name="wq", bufs=1) as wqp, \
             tc.tile_pool(name="xs", bufs=3) as xsp, \
             tc.tile_pool(name="rope_tmp", bufs=2) as rtp, \
             tc.tile_pool(name="probs", bufs=24) as prp, \
             tc.tile_pool(name="attn_sb", bufs=6) as asb:

            # ---- qkv weights first on the ACT HWDGE ring (parallel to the
            # SP ring that streams x), so the first matmul starts early ----
            w4 = []
            for g in range(4):
                wg = wqp.tile([128, 4, QKVF], bf16, tag=f"w{g}", name=f"w{g}")
                eng = nc.scalar if g % 2 == 0 else nc.gpsimd
                eng.dma_start(
                    out=wg,
                    in_=wqkvT[g * 512:(g + 1) * 512, :].rearrange(
                        "(k p) f -> p k f", p=128))
                w4.append(wg)

            # persistent small tensors (also ACT ring; needed later than w)
            cos_sb = persist.tile([HD, T], bf16, tag="cos")
            sin_sb = persist.tile([HD, T], bf16, tag="sin")
            nc.gpsimd.dma_start(out=cos_sb, in_=cosE[:])
            nc.gpsimd.dma_start(out=sin_sb, in_=sinE[:])
            md_sb = persist.tile([128, 128], bf16, tag="md")
            mw_sb = persist.tile([128, 128], bf16, tag="mw")
            nc.gpsimd.dma_start(out=md_sb, in_=mdT[:])
            nc.gpsimd.dma_start(out=mw_sb, in_=mwT[:])
            rp_sb = persist.tile([128, 128], bf16, tag="rp")
            nc.gpsimd.dma_start(out=rp_sb, in_=rotP[:])
            ident_b = persist.tile([128, 128], bf16, tag="idb")
            make_identity(nc, ident_b)
            wp_sb = persist.tile([128, NQL, C], bf16, tag="wp")
            nc.gpsimd.dma_start(out=wp_sb, in_=wpT[:].rearrange("(kd p) o -> p kd o", p=128))

            # qkv outputs (transposed layout [feature, token]), split into
            # per-512-token-chunk tiles so cross-phase deps stay chunk-local
            qTc = [[qkv_out.tile([HD, 512], bf16, tag=f"qT{h}_{n}", name=f"qT{h}_{n}")
                    for n in range(NCH)] for h in range(NQL)]
            kTc = [qkv_out.tile([HD, 512], bf16, tag=f"kT{n}", name=f"kT{n}")
                   for n in range(NCH)]
            vTc = [qkv_out.tile([HD, 512], bf16, tag=f"vT{n}", name=f"vT{n}")
                   for n in range(NCH)]
            v_sb = [qkv_out.tile([128, VW], bf16, tag=f"v{t}", name=f"v{t}")
                    for t in range(QB)]
            yTc = [[ytile.tile([HD, 512], bf16, tag=f"yT{h}_{n}", name=f"yT{h}_{n}")
                    for n in range(NCH)] for h in range(NQL)]

            def qkv_chunk(n, qkps, rops):
                ns = slice(n * 512, (n + 1) * 512)
                xc = xsp.tile([128, KC, 512], bf16, tag="x", name=f"x{n}")
                if n == 0:
                    # split the first load so matmuls start at half-arrival
                    nc.sync.dma_start(
                        out=xc[:, 0:8, :],
                        in_=xT[0:1024, ns].rearrange("(k p) t -> p k t", p=128))
                    nc.sync.dma_start(
                        out=xc[:, 8:16, :],
                        in_=xT[1024:2048, ns].rearrange("(k p) t -> p k t", p=128))
                else:
                    nc.sync.dma_start(
                        out=xc,
                        in_=xT[:, ns].rearrange("(k p) t -> p k t", p=128))
                for m in range(QKVF // 128):
                    acc = qkps.tile([128, 512], f32, tag="acc", name=f"acc{n}_{m}")
                    for k in range(KC):
                        nc.tensor.matmul(acc, w4[k // 4][:, k % 4, m * 128:(m + 1) * 128],
                                         xc[:, k, :],
                                         start=(k == 0), stop=(k == KC - 1))
                    if m < NQL:
                        nc.scalar.copy(out=qTc[m][n], in_=acc)
                    elif m == NQL:
                        nc.scalar.copy(out=kTc[n], in_=acc)
                    else:
                        nc.scalar.copy(out=vTc[n], in_=acc)
                # rope this chunk (PE rotate via signed permutation matrix)
                for th in range(NQL + 1):
                    src = qTc[th][n] if th < NQL else kTc[n]
                    rot = rops.tile([HD, 512], f32, tag="rot", name=f"rot{n}_{th}")
                    nc.tensor.matmul(rot, rp_sb, src[:, :], start=True, stop=True)
                    tmp = rtp.tile([HD, 512], bf16, tag="tmp")
                    nc.vector.tensor_mul(out=tmp, in0=rot, in1=sin_sb[:, ns])
                    nc.vector.tensor_mul(out=src[:, :], in0=src[:, :], in1=cos_sb[:, ns])
                    nc.vector.tensor_add(out=src[:, :], in0=src[:, :], in1=tmp)
                # v transpose to token-major (xbar DMA transpose, ACT ring)
                # with an appended ones-column for the softmax denominator
                for t in range(4 * n, 4 * n + 4):
                    nc.sync.dma_start_transpose(v_sb[t][:, 0:128],
                                                vTc[n][:, (t - 4 * n) * 128:(t - 4 * n + 1) * 128])
                    nc.vector.memset(v_sb[t][:, 128:VW], 1.0)

            def scores_block(h, kb, scps, split=False):
                """Transposed scores for key-block kb: sc[k, q] over the
                valid q-window [kb*128, kb*128+W)."""
                w = min(640, T - kb * 128)
                q0 = kb * 128
                # NOTE: sc spans two PSUM banks (cols 0:512 / 512:640). A
                # matmul's start=True clears has_written only for its own
                # bank, so each bank must open its own accumulation group.
                # The q-window is cut at chunk-tile boundaries (q mult of
                # 512) and at the sc bank boundary (col 512).
                lhs_k = kTc[kb // 4][:, (kb % 4) * 128:(kb % 4 + 1) * 128]
                probs = prp.tile([128, 640], bf16, tag="probs", name=f"pr{h}_{kb}")
                if split:
                    # single-bank variant (fits beside phase-1 pools):
                    # main block + separate edge tile, one exp each
                    scps_m, scps_e = scps
                    w0 = min(512, w)
                    scm = scps_m.tile([128, 512], f32, tag="scm", name=f"scm{h}_{kb}")
                    cuts = sorted({q0, q0 + w0} |
                                  set(range((q0 // 512 + 1) * 512, q0 + w0, 512)))
                    for a, b in zip(cuts[:-1], cuts[1:]):
                        nc.tensor.matmul(
                            scm[:, a - q0:b - q0], lhs_k,
                            qTc[h][a // 512][:, a % 512:a % 512 + (b - a)],
                            start=(a == q0), stop=False)
                    nc.tensor.matmul(scm[:, 0:128], ident_b, md_sb,
                                     start=False, stop=True)
                    nc.scalar.activation(out=probs[:, :w0], in_=scm[:, :w0],
                                         func=mybir.ActivationFunctionType.Exp,
                                         scale=SCALE)
                    if w > 512:
                        sce = scps_e.tile([128, 128], f32, tag="sce", name=f"sce{h}_{kb}")
                        a = q0 + 512
                        nc.tensor.matmul(sce, lhs_k,
                                         qTc[h][a // 512][:, a % 512:a % 512 + 128],
                                         start=True, stop=False)
                        nc.tensor.matmul(sce, ident_b, mw_sb, start=False, stop=True)
                        nc.scalar.activation(out=probs[:, 512:640], in_=sce,
                                             func=mybir.ActivationFunctionType.Exp,
                                             scale=SCALE)
                    return probs
                # NOTE: sc spans two PSUM banks (cols 0:512 / 512:640). A
                # matmul's start=True clears has_written only for its own
                # bank, so each bank must open its own accumulation group.
                # The q-window is cut at chunk-tile boundaries (q mult of
                # 512) and at the sc bank boundary (col 512).
                sc = scps.tile([128, 640], f32, tag="sc", name=f"sc{h}_{kb}")
                cuts = {q0, q0 + w}
                cuts.update(range((q0 // 512 + 1) * 512, q0 + w, 512))
                if q0 + 512 < q0 + w:
                    cuts.add(q0 + 512)
                cuts = sorted(cuts)
                for a, b in zip(cuts[:-1], cuts[1:]):
                    nc.tensor.matmul(
                        sc[:, a - q0:b - q0], lhs_k,
                        qTc[h][a // 512][:, a % 512:a % 512 + (b - a)],
                        start=(a - q0 in (0, 512)), stop=False)
                    if b - q0 == 512 or b == q0 + w:
                        if b - q0 <= 512:
                            nc.tensor.matmul(sc[:, 0:128], ident_b, md_sb,
                                             start=False, stop=True)
                        elif w == 640:
                            nc.tensor.matmul(sc[:, 512:640], ident_b, mw_sb,
                                             start=False, stop=True)
                nc.scalar.activation(out=probs[:, :w], in_=sc[:, :w],
                                     func=mybir.ActivationFunctionType.Exp,
                                     scale=SCALE)
                return probs

            def pv_block(h, qb, probsT, ypps):
                kt_lo = max(0, qb - 4)
                nk = qb - kt_lo + 1
                yp = ypps.tile([128, VW], f32, tag="yp", name=f"yp{h}_{qb}")
                for j in range(nk):
                    kb = kt_lo + j
                    nc.tensor.matmul(yp, probsT[kb][:, (qb - kb) * 128:(qb - kb + 1) * 128],
                                     v_sb[kb][:, 0:VW],
                                     start=(j == 0), stop=(j == nk - 1))
                rr = asb.tile([128, 1], f32, tag="rr", name=f"rr{h}_{qb}")
                nc.vector.reciprocal(rr, yp[:, HD:HD + 1])
                yb = asb.tile([128, 128], bf16, tag="yb", name=f"yb{h}_{qb}")
                nc.vector.tensor_scalar_mul(yb, yp[:, 0:HD], rr)
                ytp = ypps.tile([128, 128], bf16, tag="yp", name=f"ytp{h}_{qb}")
                nc.tensor.transpose(ytp, yb, ident_b)
                nc.vector.tensor_copy(
                    out=yTc[h][qb // 4][:, (qb % 4) * 128:(qb % 4 + 1) * 128],
                    in_=ytp)

            def proj_chunk(n, pjps, nsplit=2, t0=0, tw=512):
                ns = slice(n * 512 + t0, n * 512 + t0 + tw)
                mper = 16 // nsplit
                for part in range(nsplit):
                    os_c = outst.tile([128, mper, tw], bf16, tag="os",
                                      name=f"os{n}_{part}_{t0}")
                    for mi in range(mper):
                        mo = part * mper + mi
                        pp = pjps.tile([128, tw], f32, tag="pp",
                                       name=f"pp{mo}_{n}_{t0}")
                        for kd in range(NQL):
                            nc.tensor.matmul(pp, wp_sb[:, kd, mo * 128:(mo + 1) * 128],
                                             yTc[kd][n][:, t0:t0 + tw],
                                             start=(kd == 0), stop=(kd == NQL - 1))
                        # alternate the PSUM->SBUF evacuation between ACT and
                        # DVE so neither engine gates the projection
                        if mo % 2 == 0:
                            nc.scalar.copy(out=os_c[:, mi, :], in_=pp)
                        else:
                            nc.vector.tensor_copy(out=os_c[:, mi, :], in_=pp)
                    nc.sync.dma_start(
                        out=outT[part * mper * 128:(part + 1) * mper * 128,
                                 ns].rearrange("(mo p) t -> p mo t", p=128),
                        in_=os_c)

            # ---- phase 1: QKV + rope + v-transpose, chunk by chunk;
            # attention steps 0-2 run warm inside this scope (single-bank
            # score tiles fit the remaining banks) to cover the pool-close
            # boundary while chunk 3's rope drains ----
            NWARM = 3
            probsT = [[None] * QB for _ in range(NQL)]
            with tc.tile_pool(name="qkps", bufs=2, space="PSUM") as qkps, \
                 tc.tile_pool(name="ropeps", bufs=2, space="PSUM") as rops:
                for n in range(NCH):
                    qkv_chunk(n, qkps, rops)
                with tc.tile_pool(name="scwm", bufs=2, space="PSUM") as scwm, \
                     tc.tile_pool(name="scwe", bufs=1, space="PSUM") as scwe, \
                     tc.tile_pool(name="ypw", bufs=1, space="PSUM") as ypw:
                    for s in range(NWARM):
                        for h in range(NQL):
                            probsT[h][s] = scores_block(h, s, (scwm, scwe), split=True)
                        if _dbg and s == 0:
                            nc.sync.dma_start(out=prd[:], in_=probsT[0][0])
                        for h in range(NQL):
                            pv_block(h, s, probsT[h], ypw)

            # ---- phase 2: attention (step-major) + per-chunk projection ----
            with tc.tile_pool(name="scps", bufs=2, space="PSUM") as scps, \
                 tc.tile_pool(name="ypps", bufs=2, space="PSUM") as ypps, \
                 tc.tile_pool(name="pjps", bufs=2, space="PSUM") as pjps:
                # projection chunks fully covered by the warm steps
                for b in range((NWARM - 4) // 4 + 1 if NWARM >= 4 else 0):
                    proj_chunk(b, pjps)
                for s in range(NWARM, QB):
                    for h in range(NQL):
                        probsT[h][s] = scores_block(h, s, scps)
                    for h in range(NQL):
                        pv_block(h, s, probsT[h], ypps)
                    if s % 4 == 3 and s < QB - 1:
                        proj_chunk(s // 4, pjps)
                    elif s == QB - 3:
                        proj_chunk(NCH - 1, pjps, nsplit=2, t0=0, tw=256)
            with tc.tile_pool(name="pjps2", bufs=4, space="PSUM") as pjps2:
                proj_chunk(NCH - 1, pjps2, nsplit=4, t0=256, tw=256)

            if _dbg:
                for h in range(NQL):
                    for n in range(NCH):
                        ns = slice(n * 512, (n + 1) * 512)
                        nc.sync.dma_start(out=qkd[h * HD:(h + 1) * HD, ns], in_=qTc[h][n])
                        nc.sync.dma_start(out=yTd[h * HD:(h + 1) * HD, ns], in_=yTc[h][n])
                for n in range(NCH):
                    ns = slice(n * 512, (n + 1) * 512)
                    nc.sync.dma_start(out=qkd[4 * HD:5 * HD, ns], in_=kTc[n])
                nc.sync.dma_start(out=vd[:], in_=v_sb[0])
    nc.finalize()
    return nc


def _prep_inputs(x, w_qkv, w_proj, freqs_cos, freqs_sin):
    """Build the 8 per-core input maps (host-side shard + transpose)."""
    x = np.asarray(x, dtype=np.float32)
    w_qkv = np.asarray(w_qkv, dtype=np.float32)
    w_proj = np.asarray(w_proj, dtype=np.float32)
    freqs_cos = np.asarray(freqs_cos, dtype=np.float32)
    freqs_sin = np.asarray(freqs_sin, dtype=np.float32)
    bf = ml_dtypes.bfloat16

    # interleaved-pair rope tables expanded to [HD, T]
    cosE = np.ascontiguousarray(np.repeat(freqs_cos.T, 2, axis=0)).astype(bf)
    sinE = np.ascontiguousarray(np.repeat(freqs_sin.T, 2, axis=0)).astype(bf)
    # signed pair-rotation matrix: rot = P.T @ t, rot[2r] = -t[2r+1], rot[2r+1] = t[2r]
    rotP = np.zeros((HD, HD), np.float32)
    idx = np.arange(0, HD, 2)
    rotP[idx + 1, idx] = -1.0
    rotP[idx, idx + 1] = 1.0
    rotP = rotP.astype(bf)
    r = np.arange(128)[:, None]   # k within block
    jj = np.arange(128)[None, :]  # q within block
    mdT = np.where(jj >= r, 0.0, NEG).astype(np.float32).astype(bf)
    mwT = np.where(jj < r, 0.0, NEG).astype(np.float32).astype(bf)

    xTs = [np.ascontiguousarray(x[b].T).astype(bf) for b in range(2)]
    in_maps = []
    for c in range(8):
        b, g = divmod(c, 4)
        wq = w_qkv[g * NQL * HD:(g + 1) * NQL * HD]          # [512, C]
        wk = w_qkv[NH * HD + g * HD: NH * HD + (g + 1) * HD]  # [128, C]
        wv = w_qkv[(NH + NKV) * HD + g * HD: (NH + NKV) * HD + (g + 1) * HD]
        wqkvT = np.ascontiguousarray(
            np.concatenate([wq, wk, wv], axis=0).T).astype(bf)
        wpT = np.ascontiguousarray(
            w_proj[:, g * NQL * HD:(g + 1) * NQL * HD].T).astype(bf)
        in_maps.append({
            "xT": xTs[b], "wqkvT": wqkvT, "wpT": wpT,
            "cosE": cosE, "sinE": sinE, "mdT": mdT, "mwT": mwT,
            "rotP": rotP,
        })
    return in_maps


def _run(in_maps, trace=False):
    if "nc" not in _CACHE:
        _CACHE["nc"] = _build_program()
    return run_bass_kernel_spmd(_CACHE["nc"], in_maps, core_ids=list(range(8)),
                                trace=trace)


def kernel(x, w_qkv, w_proj, freqs_cos, freqs_sin, mask=None, _trace=False):
    in_maps = _prep_inputs(x, w_qkv, w_proj, freqs_cos, freqs_sin)
    res = _run(in_maps, trace=_trace)
    out = np.empty((2, T, C), dtype=np.float32)
    for b in range(2):
        acc = res.results[b * 4]["outT"].astype(np.float32)
        for g in range(1, 4):
            acc = acc + res.results[b * 4 + g]["outT"].astype(np.float32)
        out[b] = acc.T
    if _trace:
        return out, res
    return out

